# revision 1
# baseline (speedup 1.0000x reference)
"""HMTextCrossAttention Trainium2 kernel.

Cross-attention: out = softmax(mask((hm@Wq+bq) @ (text@Wk+bk)^T / sqrt(d))) @ (text@Wv+bv)
Shapes: B=8, Lq=Lk=2048, d=1024, fp32 inputs/outputs, int32 mask.

Distribution: pure data-parallel over batch — one batch element per NeuronCore,
8 cores, no collectives. Each core runs the identical NEFF (SPMD) on its shard.

Single-core algorithm (all matmuls on TensorE):
  - Activations are transposed on-chip via PE-transpose (fp32 has no DMA
    transpose) so the contraction dim (d) lies on SBUF partitions.
  - Projections and Q@K^T run in float32r (~fp22 mantissa, full PE rate at
    free-dim>=256); probabilities and V run in bf16 (error ~3e-3 overall).
  - Softmax is computed unnormalized: P = exp(s/32 + maskbias) where
    maskbias = (mask-1)*1e9 folds masking, scaling and exp into one ACT op
    per tile. Z comes from a ones-vector matmul; normalization is a single
    per-partition multiply of the attn@V accumulator.
  - S^T (= K @ Q^T) is produced with Lk on partitions so P^T tiles can be
    used directly as lhsT in attn@V, which writes the output in natural
    [Lq, d] layout - no output transpose needed.
"""

import numpy as np

import concourse.bacc as bacc
import concourse.mybir as mybir
import concourse.tile as tile
from concourse.bass_utils import run_bass_kernel_spmd
from concourse.masks import make_identity

F32 = mybir.dt.float32
F32R = mybir.dt.float32r
BF16 = mybir.dt.bfloat16
I32 = mybir.dt.int32
AF = mybir.ActivationFunctionType
ALU = mybir.AluOpType

B = 8
L = 2048          # Lq == Lk
D = 1024
P = 128
KO = D // P       # 8 d-subtiles
TT = L // P       # 16 L-subtiles
NB = 512          # free-dim block for matmuls / L-block size
NQ = L // NB      # 4 blocks
SCALE = float(1.0 / np.sqrt(D))   # 0.03125

_CACHE = {}


def _build():
    nc = bacc.Bacc(None, target_bir_lowering=False)

    hm = nc.dram_tensor("hm_tokens", [L, D], F32, kind="ExternalInput")
    text = nc.dram_tensor("text_tokens", [L, D], F32, kind="ExternalInput")
    mask = nc.dram_tensor("text_mask", [L], I32, kind="ExternalInput")
    Wq = nc.dram_tensor("Wq", [D, D], F32, kind="ExternalInput")
    bq = nc.dram_tensor("bq", [D], F32, kind="ExternalInput")
    Wk = nc.dram_tensor("Wk", [D, D], F32, kind="ExternalInput")
    bk = nc.dram_tensor("bk", [D], F32, kind="ExternalInput")
    Wv = nc.dram_tensor("Wv", [D, D], F32, kind="ExternalInput")
    bv = nc.dram_tensor("bv", [D], F32, kind="ExternalInput")
    out = nc.dram_tensor("out", [L, D], F32, kind="ExternalOutput")

    from contextlib import ExitStack
    with ExitStack() as ctx:
        tc = ctx.enter_context(tile.TileContext(nc))
        consts = ctx.enter_context(tc.tile_pool(name="consts", bufs=1))
        wpool = ctx.enter_context(tc.tile_pool(name="wpool", bufs=1))
        ktp = ctx.enter_context(tc.tile_pool(name="kt", bufs=1))
        vp = ctx.enter_context(tc.tile_pool(name="vp", bufs=1))
        natp = ctx.enter_context(tc.tile_pool(name="nat", bufs=2))
        actTp = ctx.enter_context(tc.tile_pool(name="actT", bufs=2))
        qtp = ctx.enter_context(tc.tile_pool(name="qt", bufs=1))
        ppp = ctx.enter_context(tc.tile_pool(name="pp", bufs=1))
        zrp = ctx.enter_context(tc.tile_pool(name="zr", bufs=2))
        outp = ctx.enter_context(tc.tile_pool(name="outp", bufs=2))
        dram = ctx.enter_context(tc.tile_pool(name="dram", bufs=1, space="DRAM"))
        ps_mm = ctx.enter_context(tc.tile_pool(name="ps_mm", bufs=2, space="PSUM"))
        ps_tr = ctx.enter_context(tc.tile_pool(name="ps_tr", bufs=2, space="PSUM"))
        ps_o = ctx.enter_context(tc.tile_pool(name="ps_o", bufs=2, space="PSUM"))
        ps_zt = ctx.enter_context(tc.tile_pool(name="ps_zt", bufs=1, space="PSUM"))
        if True:
            # ---- constants / small prep ----
            ident = consts.tile([P, P], F32)
            make_identity(nc, ident)

            mk_i = consts.tile([P, TT], I32)
            nc.sync.dma_start(mk_i[:], mask[:].rearrange("(t p) -> p t", p=P))
            maskbias = consts.tile([P, TT], F32)
            # mask 1 -> 0.0 ; mask 0 -> -1e9  (exp underflows to exactly 0)
            nc.vector.tensor_scalar(maskbias[:], mk_i[:], 1e9, -1e9, ALU.mult, ALU.add)

            biasq = consts.tile([P, KO], F32)
            nc.sync.dma_start(biasq[:], bq[:].rearrange("(o p) -> p o", p=P))
            biask = consts.tile([P, KO], F32)
            nc.sync.dma_start(biask[:], bk[:].rearrange("(o p) -> p o", p=P))

            ones_bf = consts.tile([P, 1], BF16)
            nc.vector.memset(ones_bf[:], 1.0)

            # bv broadcast to [128, D] via ones outer product (PE handles
            # partition broadcast; DVE cannot read stride-0 partition APs)
            bv1 = natp.tile([1, D], F32, tag="nat")
            nc.sync.dma_start(bv1[:], bv[:].unsqueeze(0))
            bv_bf = natp.tile([1, D], BF16, tag="nat")
            nc.vector.tensor_copy(bv_bf[:], bv1[:])
            ones1 = consts.tile([1, P], BF16)
            nc.vector.memset(ones1[:], 1.0)
            bv_bc = consts.tile([P, D], BF16)
            for nn in range(D // NB):
                pvb = ps_o.tile([P, NB], F32, tag="pso")
                nc.tensor.matmul(pvb[:], ones1[:], bv_bf[:, nn * NB:(nn + 1) * NB],
                                 start=True, stop=True)
                nc.vector.tensor_copy(bv_bc[:, nn * NB:(nn + 1) * NB], pvb[:])

            qt_dram = dram.tile([P, KO, L], F32)

            def transpose_block(src_dram, r0):
                """Load [NB, D] rows r0:r0+NB of src, return [128, KO, NB] f32r
                tile holding the transpose (d on partitions)."""
                tt = actTp.tile([P, KO, NB], F32R, tag="actT")
                for j in range(NB // P):
                    natt = natp.tile([P, D], F32, tag="nat")
                    nc.sync.dma_start(natt[:], src_dram[r0 + j * P: r0 + (j + 1) * P, :])
                    for ki in range(KO):
                        ptr = ps_tr.tile([P, P], F32, tag="pstr")
                        nc.tensor.transpose(ptr[:], natt[:, ki * P:(ki + 1) * P], ident[:])
                        nc.vector.tensor_copy(tt[:, ki, j * P:(j + 1) * P], ptr[:])
                return tt

            # ---- Phase A: Q^T = Wq^T @ hm^T + bq, spilled to DRAM scratch ----
            wq_sb = wpool.tile([P, KO, D], F32R, tag="W")
            nc.sync.dma_start(wq_sb[:], Wq[:].rearrange("(ko p) m -> p ko m", p=P).bitcast(F32R))
            for qb in range(NQ):
                hmT = transpose_block(hm, qb * NB)
                qt_sb = qtp.tile([P, KO, NB], F32R, tag="qt")
                for do in range(KO):
                    pmm = ps_mm.tile([P, NB], F32, tag="psmm")
                    for ki in range(KO):
                        nc.tensor.matmul(pmm[:], wq_sb[:, ki, do * P:(do + 1) * P],
                                         hmT[:, ki, :], start=(ki == 0), stop=(ki == KO - 1))
                    nc.scalar.activation(qt_sb[:, do, :], pmm[:], AF.Identity,
                                         bias=biasq[:, do:do + 1], scale=1.0)
                nc.sync.dma_start(qt_dram[:, :, qb * NB:(qb + 1) * NB].bitcast(F32R), qt_sb[:])

            # ---- Phase B1: K^T = Wk^T @ text^T + bk (resident, f32r) ----
            kt_sb = ktp.tile([P, KO, L], F32R)
            wk_sb = wpool.tile([P, KO, D], F32R, tag="W")
            nc.sync.dma_start(wk_sb[:], Wk[:].rearrange("(ko p) m -> p ko m", p=P).bitcast(F32R))
            for kb in range(NQ):
                ttT = transpose_block(text, kb * NB)
                for do in range(KO):
                    pmm = ps_mm.tile([P, NB], F32, tag="psmm")
                    for ki in range(KO):
                        nc.tensor.matmul(pmm[:], wk_sb[:, ki, do * P:(do + 1) * P],
                                         ttT[:, ki, :], start=(ki == 0), stop=(ki == KO - 1))
                    nc.scalar.activation(kt_sb[:, do, kb * NB:(kb + 1) * NB], pmm[:],
                                         AF.Identity, bias=biask[:, do:do + 1], scale=1.0)

            # ---- Phase B2: V = text @ Wv + bv (resident, bf16, Lk on partitions) ----
            v_sb = vp.tile([P, TT, D], BF16)
            wv_sb = wpool.tile([P, KO, D], F32R, tag="W")
            nc.sync.dma_start(wv_sb[:], Wv[:].rearrange("(ko p) m -> p ko m", p=P).bitcast(F32R))
            for kb in range(NQ):
                ttT = transpose_block(text, kb * NB)
                for j in range(NB // P):
                    t_idx = kb * (NB // P) + j
                    for nn in range(D // NB):
                        pmm = ps_mm.tile([P, NB], F32, tag="psmm")
                        for ki in range(KO):
                            nc.tensor.matmul(pmm[:], ttT[:, ki, j * P:(j + 1) * P],
                                             wv_sb[:, ki, nn * NB:(nn + 1) * NB],
                                             start=(ki == 0), stop=(ki == KO - 1))
                        nc.vector.tensor_tensor(v_sb[:, t_idx, nn * NB:(nn + 1) * NB],
                                                pmm[:], bv_bc[:, nn * NB:(nn + 1) * NB], ALU.add)

            # ---- Phase C: attention per Lq block ----
            for qb in range(NQ):
                qt_blk = qtp.tile([P, KO, NB], F32R, tag="qt")
                nc.sync.dma_start(qt_blk[:], qt_dram[:, :, qb * NB:(qb + 1) * NB].bitcast(F32R))

                # S^T tiles (Lk on partitions) -> P = exp(S/sqrt(d) + maskbias), bf16
                p_blk = ppp.tile([P, TT, NB], BF16, tag="p")
                for t in range(TT):
                    pst = ps_mm.tile([P, NB], F32, tag="psmm")
                    for di in range(KO):
                        nc.tensor.matmul(pst[:], kt_sb[:, di, t * P:(t + 1) * P],
                                         qt_blk[:, di, :], start=(di == 0), stop=(di == KO - 1))
                    nc.scalar.activation(p_blk[:, t, :], pst[:], AF.Exp,
                                         bias=maskbias[:, t:t + 1], scale=SCALE)

                # Z per-partition: Z[q] = sum_k P[k, q] via ones-rhs matmuls
                zr_sb = zrp.tile([P, NB // P], F32, tag="zr")
                for j in range(NB // P):
                    pzt = ps_zt.tile([P, 1], F32, tag="pszt")
                    for t in range(TT):
                        nc.tensor.matmul(pzt[:], p_blk[:, t, j * P:(j + 1) * P],
                                         ones_bf[:], start=(t == 0), stop=(t == TT - 1))
                    nc.vector.reciprocal(zr_sb[:, j:j + 1], pzt[:])

                # out[qb block] = (P^T)^T @ V / Z
                for j in range(NB // P):
                    for nn in range(D // NB):
                        po = ps_o.tile([P, NB], F32, tag="pso")
                        for t in range(TT):
                            nc.tensor.matmul(po[:], p_blk[:, t, j * P:(j + 1) * P],
                                             v_sb[:, t, nn * NB:(nn + 1) * NB],
                                             start=(t == 0), stop=(t == TT - 1))
                        o_sb = outp.tile([P, NB], F32, tag="o")
                        nc.vector.tensor_scalar_mul(o_sb[:], po[:], zr_sb[:, j:j + 1])
                        nc.sync.dma_start(
                            out[qb * NB + j * P: qb * NB + (j + 1) * P, nn * NB:(nn + 1) * NB],
                            o_sb[:])

    nc.compile()
    return nc


def kernel(hm_tokens, text_tokens, text_mask, Wq, bq, Wk, bk, Wv, bv):
    nc = _CACHE.get("nc")
    if nc is None:
        nc = _CACHE["nc"] = _build()

    Wq = np.ascontiguousarray(Wq, np.float32)
    bq = np.ascontiguousarray(bq, np.float32)
    Wk = np.ascontiguousarray(Wk, np.float32)
    bk = np.ascontiguousarray(bk, np.float32)
    Wv = np.ascontiguousarray(Wv, np.float32)
    bv = np.ascontiguousarray(bv, np.float32)
    in_maps = [
        {
            "hm_tokens": np.ascontiguousarray(hm_tokens[b], np.float32),
            "text_tokens": np.ascontiguousarray(text_tokens[b], np.float32),
            "text_mask": np.ascontiguousarray(text_mask[b], np.int32),
            "Wq": Wq, "bq": bq, "Wk": Wk, "bk": bk, "Wv": Wv, "bv": bv,
        }
        for b in range(B)
    ]
    res = run_bass_kernel_spmd(nc, in_maps, core_ids=list(range(B)))
    return np.stack([res.results[b]["out"] for b in range(B)]).astype(np.float32)



# revision 3
# speedup vs baseline: 44.5184x; 44.5184x over previous
"""HMTextCrossAttention Trainium2 kernel (v3).

Cross-attention: out = softmax(mask((hm@Wq+bq) @ (text@Wk+bk)^T / sqrt(d))) @ (text@Wv+bv)
Shapes: B=8, Lq=Lk=2048, d=1024, fp32 inputs/outputs, int32 mask.

Distribution: pure data-parallel over batch - one batch element per NeuronCore,
8 cores, no collectives. Each core runs the identical NEFF (SPMD) on its shard.

v2 changes vs v1:
  - All matmul operands bf16 (PSUM accumulation fp32): weights converted
    on-chip, transposed activations written bf16 straight from the PSUM
    evacuation copy. Same PE rate as f32r but halves SBUF so everything
    fits resident:
  - Q^T kept resident in SBUF (no DRAM spill round-trip).
  - K and V projections share one PE-transpose of text (was done twice).
  - Z (softmax denominator) matmuls interleaved into the attn@V t-loop so
    they share the p-tile stationary already in the PE array: the old
    separate Z pass was LDWEIGHTS-bound (~27us of PE time), now ~free.
  - PE transposes write 4-up into one PSUM bank, evacuated by one DVE copy.

v3 changes vs v2:
  - Weight loads use full row-block chunks (4KB contiguous DMA lines, was
    512B half-rate lines) and are issued on the scalar-engine HWDGE ring,
    prefetched at rep start, so they never head-of-line block the
    activation loads on the sync ring.
  - Output stores go out on the scalar-engine HWDGE ring.
  - pz (Z accumulation) matmul issues first per t so the next t's
    LDWEIGHTS hides under the two long N=512 matmuls, not the N=1 one.
"""

import numpy as np

import concourse.bacc as bacc
import concourse.mybir as mybir
import concourse.tile as tile
from concourse.bass_utils import run_bass_kernel_spmd
from concourse.masks import make_identity

F32 = mybir.dt.float32
BF16 = mybir.dt.bfloat16
I32 = mybir.dt.int32
AF = mybir.ActivationFunctionType
ALU = mybir.AluOpType

B = 8
L = 2048          # Lq == Lk
D = 1024
P = 128
KO = D // P       # 8 d-subtiles
TT = L // P       # 16 L-subtiles
NB = 512          # free-dim block for matmuls / L-block size
NQ = L // NB      # 4 blocks
WC = 128          # weight-conversion chunk (columns)
SCALE = float(1.0 / np.sqrt(D))   # 0.03125

_CACHE = {}


def _build(reps=1, phases="abc"):
    nc = bacc.Bacc(None, target_bir_lowering=False)

    hm = nc.dram_tensor("hm_tokens", [L, D], F32, kind="ExternalInput")
    text = nc.dram_tensor("text_tokens", [L, D], F32, kind="ExternalInput")
    mask = nc.dram_tensor("text_mask", [L], I32, kind="ExternalInput")
    Wq = nc.dram_tensor("Wq", [D, D], F32, kind="ExternalInput")
    bq = nc.dram_tensor("bq", [D], F32, kind="ExternalInput")
    Wk = nc.dram_tensor("Wk", [D, D], F32, kind="ExternalInput")
    bk = nc.dram_tensor("bk", [D], F32, kind="ExternalInput")
    Wv = nc.dram_tensor("Wv", [D, D], F32, kind="ExternalInput")
    bv = nc.dram_tensor("bv", [D], F32, kind="ExternalInput")
    out = nc.dram_tensor("out", [L, D], F32, kind="ExternalOutput")

    from contextlib import ExitStack
    with ExitStack() as ctx:
        tc = ctx.enter_context(tile.TileContext(nc))
        consts = ctx.enter_context(tc.tile_pool(name="consts", bufs=1))
        wstp = ctx.enter_context(tc.tile_pool(name="wst", bufs=2))
        wqp = ctx.enter_context(tc.tile_pool(name="wq", bufs=1))
        wkp = ctx.enter_context(tc.tile_pool(name="wk", bufs=1))
        wvp = ctx.enter_context(tc.tile_pool(name="wv", bufs=1))
        ktp = ctx.enter_context(tc.tile_pool(name="kt", bufs=1))
        qtp = ctx.enter_context(tc.tile_pool(name="qt", bufs=1))
        vp = ctx.enter_context(tc.tile_pool(name="vp", bufs=1))
        natp = ctx.enter_context(tc.tile_pool(name="nat", bufs=3))
        actTp = ctx.enter_context(tc.tile_pool(name="actT", bufs=2))
        ppp = ctx.enter_context(tc.tile_pool(name="pp", bufs=1))
        zrp = ctx.enter_context(tc.tile_pool(name="zr", bufs=2))
        outp = ctx.enter_context(tc.tile_pool(name="outp", bufs=2))
        ps_mm = ctx.enter_context(tc.tile_pool(name="ps_mm", bufs=2, space="PSUM"))
        ps_a = ctx.enter_context(tc.tile_pool(name="ps_a", bufs=2, space="PSUM"))
        ps_b = ctx.enter_context(tc.tile_pool(name="ps_b", bufs=2, space="PSUM"))

        # ---- constants / small prep ----
        ident = consts.tile([P, P], F32)
        make_identity(nc, ident)

        mk_i = consts.tile([P, TT], I32)
        nc.sync.dma_start(mk_i[:], mask[:].rearrange("(t p) -> p t", p=P))
        maskbias = consts.tile([P, TT], F32)
        # mask 1 -> 0.0 ; mask 0 -> -1e9  (exp underflows to exactly 0)
        nc.vector.tensor_scalar(maskbias[:], mk_i[:], 1e9, -1e9, ALU.mult, ALU.add)

        biasq = consts.tile([P, KO], F32)
        nc.sync.dma_start(biasq[:], bq[:].rearrange("(o p) -> p o", p=P))
        biask = consts.tile([P, KO], F32)
        nc.sync.dma_start(biask[:], bk[:].rearrange("(o p) -> p o", p=P))

        ones_bf = consts.tile([P, 1], BF16)
        nc.vector.memset(ones_bf[:], 1.0)

        # bv broadcast to [128, D] via ones outer product (PE handles
        # partition broadcast; DVE cannot read stride-0 partition APs)
        bv1 = natp.tile([1, D], F32, tag="nat")
        nc.sync.dma_start(bv1[:], bv[:].unsqueeze(0))
        bv_bf = natp.tile([1, D], BF16, tag="nat")
        nc.vector.tensor_copy(bv_bf[:], bv1[:])
        ones1 = consts.tile([1, P], BF16)
        nc.vector.memset(ones1[:], 1.0)
        bv_bc = consts.tile([P, D], BF16)
        for nn in range(D // NB):
            pvb = ps_a.tile([P, NB], F32, tag="a")
            nc.tensor.matmul(pvb[:], ones1[:], bv_bf[:, nn * NB:(nn + 1) * NB],
                             start=True, stop=True)
            nc.vector.tensor_copy(bv_bc[:, nn * NB:(nn + 1) * NB], pvb[:])

        def load_weight_bf(wdram, dst_pool, name):
            """DMA fp32 weight row-blocks (contiguous 4KB lines), DVE-convert
            to bf16 [P, KO, D] layout: w[p, ko, m] = W[ko*128+p, m]."""
            wbf = dst_pool.tile([P, KO, D], BF16, tag=name, name=name)
            for ko in range(KO):
                wst = wstp.tile([P, D], F32, tag="wst", name="wst")
                nc.scalar.dma_start(wst[:], wdram[ko * P:(ko + 1) * P, :])
                nc.vector.tensor_copy(wbf[:, ko, :], wst[:])
            return wbf

        def transpose_block(src_dram, r0):
            """Load [NB, D] rows r0:r0+NB of src, return [128, KO, NB] bf16
            tile holding the transpose (d on partitions)."""
            tt = actTp.tile([P, KO, NB], BF16, tag="actT")
            for j in range(NB // P):
                natt = natp.tile([P, D], F32, tag="nat")
                nc.sync.dma_start(natt[:], src_dram[r0 + j * P: r0 + (j + 1) * P, :])
                for g in range(2):
                    ptr = ps_mm.tile([P, NB], F32, tag="tr")
                    for kk in range(4):
                        ki = g * 4 + kk
                        nc.tensor.transpose(ptr[:, kk * P:(kk + 1) * P],
                                            natt[:, ki * P:(ki + 1) * P], ident[:])
                    nc.vector.tensor_copy(
                        tt[:, g * 4:(g + 1) * 4, j * P:(j + 1) * P],
                        ptr[:].rearrange("p (a b) -> p a b", a=4))
            return tt

        qt_sb = qtp.tile([P, KO, L], BF16, tag="qt")
        kt_sb = ktp.tile([P, KO, L], BF16, tag="kt")
        v_sb = vp.tile([P, TT, D], BF16, tag="v")
        for rep in range(reps):
            do_a = "a" in phases or rep == 0
            do_b = "b" in phases or rep == 0
            do_c = "c" in phases or rep == 0
            wq_bf = load_weight_bf(Wq, wqp, "wqbf")
            wk_bf = load_weight_bf(Wk, wkp, "wkbf")
            wv_bf = load_weight_bf(Wv, wvp, "wvbf")

            # ---- Phase A: Q^T = Wq^T @ hm^T + bq (resident, bf16) ----
            for qb in range(NQ if do_a else 0):
                hmT = transpose_block(hm, qb * NB)
                for do in range(KO):
                    pmm = ps_mm.tile([P, NB], F32, tag="mm")
                    for ki in range(KO):
                        nc.tensor.matmul(pmm[:], wq_bf[:, ki, do * P:(do + 1) * P],
                                         hmT[:, ki, :], start=(ki == 0), stop=(ki == KO - 1))
                    nc.scalar.activation(qt_sb[:, do, qb * NB:(qb + 1) * NB], pmm[:],
                                         AF.Identity, bias=biasq[:, do:do + 1], scale=1.0)

            # ---- Phase B: K^T and V from one text^T transpose ----
            for kb in range(NQ if do_b else 0):
                ttT = transpose_block(text, kb * NB)
                # K^T projection
                for do in range(KO):
                    pmm = ps_mm.tile([P, NB], F32, tag="mm")
                    for ki in range(KO):
                        nc.tensor.matmul(pmm[:], wk_bf[:, ki, do * P:(do + 1) * P],
                                         ttT[:, ki, :], start=(ki == 0), stop=(ki == KO - 1))
                    nc.scalar.activation(kt_sb[:, do, kb * NB:(kb + 1) * NB], pmm[:],
                                         AF.Identity, bias=biask[:, do:do + 1], scale=1.0)
                # V projection (natural layout, Lk on partitions)
                for j in range(NB // P):
                    t_idx = kb * (NB // P) + j
                    pva = ps_a.tile([P, NB], F32, tag="a")
                    pvb = ps_b.tile([P, NB], F32, tag="b")
                    for ki in range(KO):
                        nc.tensor.matmul(pva[:], ttT[:, ki, j * P:(j + 1) * P],
                                         wv_bf[:, ki, 0:NB],
                                         start=(ki == 0), stop=(ki == KO - 1))
                        nc.tensor.matmul(pvb[:], ttT[:, ki, j * P:(j + 1) * P],
                                         wv_bf[:, ki, NB:D],
                                         start=(ki == 0), stop=(ki == KO - 1))
                    nc.vector.tensor_tensor(v_sb[:, t_idx, 0:NB], pva[:],
                                            bv_bc[:, 0:NB], ALU.add)
                    nc.vector.tensor_tensor(v_sb[:, t_idx, NB:D], pvb[:],
                                            bv_bc[:, NB:D], ALU.add)

            # ---- Phase C: attention per Lq block ----
            for qb in range(NQ if do_c else 0):
                # S^T tiles (Lk on partitions) -> P = exp(S/sqrt(d) + maskbias), bf16
                p_blk = ppp.tile([P, TT, NB], BF16, tag="p")
                for t in range(TT):
                    pst = ps_mm.tile([P, NB], F32, tag="mm")
                    for di in range(KO):
                        nc.tensor.matmul(pst[:], kt_sb[:, di, t * P:(t + 1) * P],
                                         qt_sb[:, di, qb * NB:(qb + 1) * NB],
                                         start=(di == 0), stop=(di == KO - 1))
                    nc.scalar.activation(p_blk[:, t, :], pst[:], AF.Exp,
                                         bias=maskbias[:, t:t + 1], scale=SCALE)

                # out[qb block] = (P^T)^T @ V / Z ; Z rides along as an N=1
                # matmul sharing the p-tile stationary.
                for j in range(NB // P):
                    poa = ps_a.tile([P, NB], F32, tag="a")
                    pob = ps_b.tile([P, NB], F32, tag="b")
                    pz = ps_mm.tile([P, 1], F32, tag="tr")
                    for t in range(TT):
                        st, sp = (t == 0), (t == TT - 1)
                        nc.tensor.matmul(pz[:], p_blk[:, t, j * P:(j + 1) * P],
                                         ones_bf[:], start=st, stop=sp)
                        nc.tensor.matmul(poa[:], p_blk[:, t, j * P:(j + 1) * P],
                                         v_sb[:, t, 0:NB], start=st, stop=sp)
                        nc.tensor.matmul(pob[:], p_blk[:, t, j * P:(j + 1) * P],
                                         v_sb[:, t, NB:D], start=st, stop=sp)
                    zr = zrp.tile([P, 1], F32, tag="zr")
                    nc.vector.reciprocal(zr[:], pz[:])
                    r0 = qb * NB + j * P
                    o1 = outp.tile([P, NB], F32, tag="o")
                    nc.vector.tensor_scalar_mul(o1[:], poa[:], zr[:])
                    nc.scalar.dma_start(out[r0:r0 + P, 0:NB], o1[:])
                    o2 = outp.tile([P, NB], F32, tag="o")
                    nc.vector.tensor_scalar_mul(o2[:], pob[:], zr[:])
                    nc.scalar.dma_start(out[r0:r0 + P, NB:D], o2[:])

    nc.compile()
    return nc


def kernel(hm_tokens, text_tokens, text_mask, Wq, bq, Wk, bk, Wv, bv):
    nc = _CACHE.get("nc")
    if nc is None:
        nc = _CACHE["nc"] = _build()

    Wq = np.ascontiguousarray(Wq, np.float32)
    bq = np.ascontiguousarray(bq, np.float32)
    Wk = np.ascontiguousarray(Wk, np.float32)
    bk = np.ascontiguousarray(bk, np.float32)
    Wv = np.ascontiguousarray(Wv, np.float32)
    bv = np.ascontiguousarray(bv, np.float32)
    in_maps = [
        {
            "hm_tokens": np.ascontiguousarray(hm_tokens[b], np.float32),
            "text_tokens": np.ascontiguousarray(text_tokens[b], np.float32),
            "text_mask": np.ascontiguousarray(text_mask[b], np.int32),
            "Wq": Wq, "bq": bq, "Wk": Wk, "bk": bk, "Wv": Wv, "bv": bv,
        }
        for b in range(B)
    ]
    res = run_bass_kernel_spmd(nc, in_maps, core_ids=list(range(B)))
    return np.stack([res.results[b]["out"] for b in range(B)]).astype(np.float32)
